# revision 28
# baseline (speedup 1.0000x reference)
"""GCN encoder (3-layer) Trainium2 kernel, 8-core SPMD.

Strategy:
  out = A @ (X @ W) per layer, A = normalized adjacency (fixed across layers).
  - Nodes sharded by contiguous id range across 8 cores (dst partition).
  - Edge slots are region-packed per (window-group, src-chunk): each window's
    slot count is the max edge count over the 8 cores (SPMD shapes), regions
    rounded to 128. 128-slot blocks straddling window boundaries get one
    matmul per overlapped window with a host-zero-masked one-hot tile.
  - Layer 0: the X rows are pre-gathered on host in slot order and streamed
    (transposed) as [128, TOTSLOT] bf16; X@W0 is computed per 128-slot block
    on PE into PSUM, copied into the reg-block layout by DVE. No runtime
    gather and no AllGather for layer 0.
  - Layers 1-2: dense transform fused into the previous layer's window loop,
    AllGather of the [NTAB,128] bf16 table, then per (group,chunk) a
    dma_gather (4 SWDGE queues, one per chunk) pulls slot rows; aggregation
    is a one-hot matmul segment-sum into PSUM (output [feat, dst-window]).
  - Table rows are laid out slice-major ([slice][core][row]) so the
    AllGather is split into 4 slices, each overlapping the tail of the
    producing layer's window loop.
  - Layer 3 output reordered: out = (A @ (X2 @ W2pad)) + b2, W2 zero-padded
    64->128 so gather rows stay 256B.
All graph structure is computed from the inputs at call time and baked into
the compiled program; counts are maxed across cores so all 8 cores run one
SPMD program.
"""

import math
import os
import numpy as np
from contextlib import ExitStack

from ml_dtypes import bfloat16

import concourse.bass as bass
import concourse.bacc as bacc
import concourse.mybir as mybir
import concourse.tile as tile
from concourse.bass_utils import run_bass_kernel_spmd
from concourse import library_config

F32 = mybir.dt.float32
BF16 = mybir.dt.bfloat16
I16 = mybir.dt.int16

NCORES = 8
D = 128          # feature width (layers 0/1/2 input, gather row width)
DOUT = 64        # final output width
WIN = 48         # dst nodes per window (one-hot width, psum free dim)
NCHUNK = 4       # src chunks (int16 gather index range)
GRP = 8          # windows per group (gather-call granularity)
NSLICE = 4       # allgather slices (groups per slice = ceil(NG/NSLICE))


class Plan:
    """Host-derived, core-invariant schedule + per-core tensor data."""

    def __init__(self, n_nodes, edge_src, edge_dst, edge_norm):
        self.N = n_nodes
        self.WPC = math.ceil(n_nodes / (NCORES * WIN))   # windows per core
        self.SH = self.WPC * WIN                         # table rows per core
        self.NTAB = NCORES * self.SH
        self.NG = math.ceil(self.WPC / GRP)
        self.groups = [list(range(g * GRP, min((g + 1) * GRP, self.WPC)))
                       for g in range(self.NG)]

        # allgather slices == src chunks: contiguous runs of groups;
        # slice-major table rows. Each slice is its own Shared tensor and
        # the int16 gather index is relative to the slice start.
        gps = self.NG // NSLICE                          # groups per slice
        assert NSLICE == NCHUNK
        bounds = [0] + [gps * (s + 1) for s in range(NSLICE - 1)] + [self.NG]
        self.slices = [list(range(bounds[s], bounds[s + 1]))
                       for s in range(NSLICE)]
        self.slice_of_group = {}
        srows = []
        for si, gs in enumerate(self.slices):
            rows = sum(len(self.groups[g]) * WIN for g in gs)
            srows.append(rows)
            for g in gs:
                self.slice_of_group[g] = si
        self.srows = srows                               # local rows per slice
        self.sstart = np.concatenate([[0], np.cumsum(srows)[:-1]]).astype(np.int64)
        self.tabsize = [r * NCORES for r in srows]       # table rows per slice
        assert all(t <= 32768 for t in self.tabsize), self.tabsize
        tabstart = np.concatenate([[0], np.cumsum(self.tabsize)[:-1]])
        self.tabstart = tabstart.astype(np.int64)        # table row of slice s

        # R[node] = table row (slice-major: [slice][core][row-in-slice])
        node = np.arange(self.NTAB, dtype=np.int64)
        k = node // self.SH
        loc = node % self.SH
        sidx = np.searchsorted(self.sstart, loc, side="right") - 1
        off = loc - self.sstart[sidx]
        L = np.array(srows, dtype=np.int64)
        self.R = self.tabstart[sidx] + k * L[sidx] + off

        core = edge_dst // self.SH
        wloc = (edge_dst % self.SH) // WIN
        dloc = edge_dst % WIN
        srow = self.R[edge_src]
        ch = (np.searchsorted(self.tabstart, srow, side="right") - 1).astype(np.int64)

        # per-(core,window,chunk) counts -> per-(window,chunk) SPMD max
        cell = (core * self.WPC + wloc) * NCHUNK + ch
        counts = np.bincount(cell, minlength=NCORES * self.WPC * NCHUNK)
        self.counts = counts.reshape(NCORES, self.WPC, NCHUNK)
        maxc = self.counts.max(axis=0)                   # [WPC, NCHUNK]

        # slot layout: for g: for ch: for w in g: maxc[w,ch] slots;
        # each (g,ch) region padded to a multiple of 128.
        self.wstart = {}       # (w, c) -> global slot start
        self.rstart = [[0] * NCHUNK for _ in range(self.NG)]
        self.rslot = [[0] * NCHUNK for _ in range(self.NG)]
        off = 0
        for g, ws in enumerate(self.groups):
            for c in range(NCHUNK):
                r0 = off
                for w in ws:
                    self.wstart[(w, c)] = off
                    off += int(maxc[w, c])
                off = -(-off // 128) * 128
                self.rstart[g][c] = r0
                self.rslot[g][c] = off - r0
        self.TOTSLOT = off
        assert self.TOTSLOT % 128 == 0

        # matmul schedule: per group, per window, list of (c, j_local, mm)
        self.sched = []        # [g][w-local] -> list of (c, j_local, mm)
        self.gmm0 = []         # per-group first mm index
        self.mmof = {}         # (w, c, j_local) -> mm
        mm = 0
        for g, ws in enumerate(self.groups):
            self.gmm0.append(mm)
            gs = []
            for w in ws:
                entries = []
                for c in range(NCHUNK):
                    n = int(maxc[w, c])
                    if n == 0:
                        continue
                    s0 = self.wstart[(w, c)]
                    j0 = (s0 - self.rstart[g][c]) // 128
                    j1 = (s0 + n - 1 - self.rstart[g][c]) // 128
                    for j in range(j0, j1 + 1):
                        entries.append((c, j, mm))
                        self.mmof[(w, c, j)] = mm
                        mm += 1
                gs.append(entries)
            self.sched.append(gs)
        self.NMM = mm
        self.gmm0.append(mm)

        self.maxc = maxc
        self.edge_core = core
        self.edge_w = wloc
        self.edge_d = dloc
        self.edge_ch = ch
        self.edge_srow = srow
        self.edge_norm = edge_norm

    def core_slots(self, k):
        """Per-core slot assignment: returns (slot, srow, w, ch, d, norm)."""
        sel = np.nonzero(self.edge_core == k)[0]
        w = self.edge_w[sel]
        ch = self.edge_ch[sel]
        d = self.edge_d[sel]
        srow = self.edge_srow[sel]
        norm = self.edge_norm[sel]
        order = np.lexsort((ch, w))
        w, ch, d, srow, norm = w[order], ch[order], d[order], srow[order], norm[order]
        bucket = w * NCHUNK + ch
        nb = self.WPC * NCHUNK
        bc = np.bincount(bucket, minlength=nb)
        starts = np.concatenate([[0], np.cumsum(bc)[:-1]])
        rank = np.arange(len(sel)) - starts[bucket]
        base = np.empty(nb, dtype=np.int64)
        for wi in range(self.WPC):
            for ci in range(NCHUNK):
                base[wi * NCHUNK + ci] = self.wstart.get((wi, ci), 0)
        slot = base[bucket] + rank
        return slot, srow, w, ch, d, norm

    def core_tensors(self, k):
        """Build idx table + one-hot table for core k."""
        slot, srow, w, ch, d, norm = self.core_slots(k)

        idxs = np.zeros(self.TOTSLOT, dtype=np.int16)
        idxs[slot] = (srow - self.tabstart[ch]).astype(np.int16)
        iw = idxs.reshape(-1, 16).T.copy()            # [16, TOTSLOT//16]
        iw = np.tile(iw, (8, 1))                      # replicate to 128 parts

        # compact one-hot: per (slot-row, mm) a (d, norm) pair; the kernel
        # expands to [128, nmm, WIN] on DVE via is_equal(iota, d) * norm.
        gidx = w // GRP
        rs = np.array([[self.rstart[g][c] for c in range(NCHUNK)]
                       for g in range(self.NG)], dtype=np.int64)
        jloc = (slot - rs[gidx, ch]) // 128
        mmv = np.empty(len(slot), dtype=np.int64)
        for i in range(len(slot)):
            mmv[i] = self.mmof[(int(w[i]), int(ch[i]), int(jloc[i]))]
        dcol = np.zeros((128, self.NMM), dtype=np.float32)
        ncol = np.zeros((128, self.NMM), dtype=np.float32)
        dcol[slot % 128, mmv] = d
        ncol[slot % 128, mmv] = norm
        return iw, dcol.astype(bfloat16), ncol.astype(bfloat16)


def _build_nc(plan, enable_asserts=False):
    p = plan
    no_coll = bool(int(os.environ.get("GCN_NO_COLL", "0")))
    nc = bacc.Bacc(
        "TRN2",
        target_bir_lowering=False,
        debug=False,
        enable_asserts=enable_asserts,
        num_devices=NCORES,
        num_swdge_queues=4,
        dynamic_dma_scratch_size=16384,
    )
    xgT = nc.dram_tensor("xgT", [D, p.TOTSLOT], BF16, kind="ExternalInput")
    w0 = nc.dram_tensor("w0", [D, D], BF16, kind="ExternalInput")
    w1 = nc.dram_tensor("w1", [D, D], BF16, kind="ExternalInput")
    w2 = nc.dram_tensor("w2", [D, D], BF16, kind="ExternalInput")
    b0 = nc.dram_tensor("b0", [D, 1], F32, kind="ExternalInput")
    b1 = nc.dram_tensor("b1", [D, 1], F32, kind="ExternalInput")
    b2 = nc.dram_tensor("b2", [D, 1], F32, kind="ExternalInput")
    idxt = nc.dram_tensor("idxt", [128, p.TOTSLOT // 16], I16, kind="ExternalInput")
    dcolt = nc.dram_tensor("dcolt", [128, p.NMM], BF16, kind="ExternalInput")
    ncolt = nc.dram_tensor("ncolt", [128, p.NMM], BF16, kind="ExternalInput")
    iotat = nc.dram_tensor("iotat", [128, WIN], BF16, kind="ExternalInput")
    identt = nc.dram_tensor("identt", [64, 64], F32, kind="ExternalInput")
    outp = nc.dram_tensor("outp", [p.SH, DOUT], F32, kind="ExternalOutput")

    with tile.TileContext(nc) as tc, ExitStack() as ctx:
        nc.gpsimd.load_library(library_config.mlp)
        sb = ctx.enter_context(tc.tile_pool(name="sb", bufs=2))
        sbg = ctx.enter_context(tc.tile_pool(name="sbg", bufs=6))
        sbo = ctx.enter_context(tc.tile_pool(name="sbo", bufs=2))
        sbh = ctx.enter_context(tc.tile_pool(name="sbh", bufs=2))
        sbp = ctx.enter_context(tc.tile_pool(name="sbp", bufs=1))
        ps = ctx.enter_context(tc.tile_pool(name="ps", bufs=2, space="PSUM"))
        psa = ctx.enter_context(tc.tile_pool(name="psa", bufs=3, space="PSUM"))
        psd = ctx.enter_context(tc.tile_pool(name="psd", bufs=3, space="PSUM"))
        dram = ctx.enter_context(tc.tile_pool(name="dram", bufs=1, space="DRAM"))

        t_outs = [dram.tile([r, D], BF16, name=f"t_out{s}")
                  for s, r in enumerate(p.srows)]
        t_tabs = [[dram.tile([p.tabsize[s], D], BF16, addr_space="Shared",
                             name=f"t_tab{i}_{s}") for s in range(len(p.srows))]
                  for i in range(2)]

        # persistent tiles
        idx_sb = sbp.tile([128, p.TOTSLOT // 16], I16)
        nc.sync.dma_start(out=idx_sb[:], in_=idxt[:])
        wt = []
        for wsrc in (w0, w1, w2):
            w_sb = sbp.tile([D, D], BF16, name=f"w_{wsrc.name}")
            nc.sync.dma_start(out=w_sb[:], in_=wsrc[:])
            wt.append(w_sb)
        bt = []
        for bsrc in (b0, b1, b2):
            b_sb = sbp.tile([D, 1], F32, name=f"b_{bsrc.name}")
            nc.sync.dma_start(out=b_sb[:], in_=bsrc[:])
            bt.append(b_sb)
        ident = sbp.tile([64, 64], F32)
        nc.sync.dma_start(out=ident[:], in_=identt[:])
        iota = sbp.tile([128, 1, WIN], BF16)
        nc.sync.dma_start(out=iota[:], in_=iotat[:])

        def do_allgather(s, layer):
            src_tile = t_outs[s]
            dst = t_tabs[layer][s]
            if no_coll:
                nc.sync.dma_start(out=dst[:p.srows[s], :], in_=src_tile[:])
                return
            nc.gpsimd.collective_compute(
                "AllGather",
                mybir.AluOpType.bypass,
                replica_groups=[list(range(NCORES))],
                ins=[src_tile.opt()],
                outs=[dst.opt()],
            )

        for layer in range(3):
            t_tab = t_tabs[layer - 1] if layer > 0 else None
            for g, ws in enumerate(p.groups):
                si = p.slice_of_group[g]
                regs = []
                for c in range(NCHUNK):
                    nslots = p.rslot[g][c]
                    nb = nslots // 128
                    pool = sbh if layer == 0 else sbg
                    reg = pool.tile([128, nb, 128], BF16,
                                    tag=f"reg{c}", name=f"reg_{layer}_{g}_{c}")
                    if layer == 0:
                        s0 = p.rstart[g][c]
                        xg = sb.tile([128, nslots], BF16, tag="xg",
                                     name=f"xg_{g}_{c}")
                        nc.sync.dma_start(out=xg[:], in_=xgT[:, s0:s0 + nslots])
                        for j0 in range(0, nb, 4):
                            j1 = min(j0 + 4, nb)
                            pd = psd.tile([128, 4, D], F32, tag="pdense",
                                          name=f"pd_{g}_{c}_{j0}")
                            for j in range(j0, j1):
                                nc.tensor.matmul(
                                    out=pd[:, j - j0, :],
                                    lhsT=xg[:, j * 128:(j + 1) * 128],
                                    rhs=wt[0][:], start=True, stop=True)
                            nc.scalar.activation(
                                out=reg[:, j0:j1, :], in_=pd[:, :j1 - j0, :],
                                func=mybir.ActivationFunctionType.Copy)
                    else:
                        ic0 = p.rstart[g][c] // 16
                        nc.gpsimd.dma_gather(
                            out_ap=reg[:],
                            in_ap=t_tab[c][:],
                            idxs_ap=idx_sb[:, ic0:ic0 + nslots // 16],
                            num_idxs=nslots,
                            num_idxs_reg=nslots,
                            elem_size=D,
                            single_packet=False,
                            queue_num=c,
                        )
                    regs.append(reg)
                mm0, mm1 = p.gmm0[g], p.gmm0[g + 1]
                nmm = mm1 - mm0
                dn_g = sbo.tile([128, 2, nmm], BF16, tag="dn",
                                name=f"dn_{layer}_{g}")
                nc.sync.dma_start(out=dn_g[:, 0, :], in_=dcolt[:, mm0:mm1])
                nc.sync.dma_start(out=dn_g[:, 1, :], in_=ncolt[:, mm0:mm1])
                oh_g = sbo.tile([128, nmm, WIN], BF16, tag="oh",
                                name=f"oh_{layer}_{g}")
                nc.vector.tensor_tensor(
                    out=oh_g[:],
                    in0=iota[:].broadcast_to([128, nmm, WIN]),
                    in1=dn_g[:, 0, :].broadcast_to([128, nmm, WIN]),
                    op=mybir.AluOpType.is_equal)
                nc.vector.tensor_tensor(
                    out=oh_g[:], in0=oh_g[:],
                    in1=dn_g[:, 1, :].broadcast_to([128, nmm, WIN]),
                    op=mybir.AluOpType.mult)
                if layer < 2:
                    tt2g = sbo.tile([WIN, GRP, D], BF16, tag="tnext",
                                    name=f"tnext_{layer}_{g}")
                else:
                    oo_g = sbo.tile([WIN, GRP, DOUT], F32, tag="oo",
                                    name=f"oo_{g}")
                for wi, w in enumerate(ws):
                    entries = p.sched[g][wi]
                    pT = psa.tile([D, WIN], F32, tag="pagg", name=f"pagg_{layer}_{w}")
                    nmmw = len(entries)
                    for k, (c, j, mmi) in enumerate(entries):
                        nc.tensor.matmul(
                            out=pT[:],
                            lhsT=regs[c][:, j, :],
                            rhs=oh_g[:, mmi - mm0, :],
                            start=(k == 0),
                            stop=(k == nmmw - 1),
                        )
                    if layer < 2:
                        xnT = sb.tile([D, WIN], BF16, tag="xn",
                                      name=f"xn_{layer}_{w}")
                        nc.scalar.activation(
                            out=xnT[:], in_=pT[:],
                            func=mybir.ActivationFunctionType.Relu,
                            bias=bt[layer][:],
                        )
                        pt2 = ps.tile([WIN, D], F32, tag="pnext",
                                      name=f"pnext_{layer}_{w}")
                        nc.tensor.matmul(out=pt2[:], lhsT=xnT[:],
                                         rhs=wt[layer + 1][:],
                                         start=True, stop=True)
                        nc.scalar.activation(
                            out=tt2g[:, wi, :], in_=pt2[:],
                            func=mybir.ActivationFunctionType.Copy)
                    else:
                        x3 = sb.tile([D, WIN], F32, tag="x3", name=f"x3_{w}")
                        nc.vector.tensor_scalar(
                            out=x3[:], in0=pT[:], scalar1=bt[2][:], scalar2=None,
                            op0=mybir.AluOpType.add)
                        po = ps.tile([WIN, DOUT], F32, tag="pnext",
                                     name=f"pout_{w}")
                        nc.tensor.matmul(out=po[:], lhsT=x3[:DOUT, :],
                                         rhs=ident[:], start=True, stop=True)
                        nc.scalar.activation(
                            out=oo_g[:, wi, :], in_=po[:],
                            func=mybir.ActivationFunctionType.Copy)
                if layer < 2:
                    # batched table write for this group's WIN*len(ws) rows
                    g0 = p.slices[si][0]
                    row0 = sum(len(p.groups[gg]) * WIN
                               for gg in p.slices[si] if gg < g)
                    nw = len(ws)
                    nc.sync.dma_start(
                        out=t_outs[si][row0:row0 + nw * WIN, :].rearrange(
                            "(w q) d -> q w d", q=WIN),
                        in_=tt2g[:, :nw, :])
                    if g == p.slices[si][-1]:
                        do_allgather(si, layer)
                else:
                    r0 = ws[0] * WIN
                    nw = len(ws)
                    nc.sync.dma_start(
                        out=outp[r0:r0 + nw * WIN, :].rearrange(
                            "(w q) f -> q w f", q=WIN),
                        in_=oo_g[:, :nw, :])
    nc.compile()
    return nc


def _prep(x, edge_index, edge_weight):
    N = x.shape[0]
    src = np.asarray(edge_index[0], dtype=np.int64)
    dst = np.asarray(edge_index[1], dtype=np.int64)
    w = np.asarray(edge_weight, dtype=np.float64)
    loop = np.arange(N, dtype=np.int64)
    src_f = np.concatenate([src, loop])
    dst_f = np.concatenate([dst, loop])
    w_f = np.concatenate([w, np.ones(N)])
    deg = np.bincount(dst_f, weights=w_f, minlength=N)
    dis = np.where(deg > 0, 1.0 / np.sqrt(np.where(deg > 0, deg, 1.0)), 0.0)
    norm = (dis[src_f] * w_f * dis[dst_f]).astype(np.float32)
    return src_f.astype(np.int64), dst_f.astype(np.int64), norm


def kernel(x, edge_index, edge_weight, W0, b0, W1, b1, W2, b2):
    x = np.asarray(x, dtype=np.float32)
    N = x.shape[0]
    src_f, dst_f, norm = _prep(x, edge_index, edge_weight)
    plan = Plan(N, src_f, dst_f, norm)

    W2p = np.zeros((D, D), dtype=np.float32)
    W2p[:, :DOUT] = np.asarray(W2)
    b2p = np.zeros((D, 1), dtype=np.float32)
    b2p[:DOUT, 0] = np.asarray(b2)

    # X rows in table-row space (R is a permutation of [0, NTAB))
    xrows = np.zeros((plan.NTAB, D), dtype=np.float32)
    xrows[plan.R[:N]] = x

    iotav = np.tile(np.arange(WIN, dtype=np.float32), (128, 1)).astype(bfloat16)
    in_maps = []
    for c in range(NCORES):
        iw, dcol, ncol = plan.core_tensors(c)
        slot, srow, _, _, _, _ = plan.core_slots(c)
        srcs = np.zeros(plan.TOTSLOT, dtype=np.int64)
        srcs[slot] = srow
        xg = np.ascontiguousarray(xrows[srcs].T).astype(bfloat16)
        in_maps.append({
            "xgT": xg,
            "w0": np.asarray(W0).astype(bfloat16),
            "w1": np.asarray(W1).astype(bfloat16),
            "w2": W2p.astype(bfloat16),
            "b0": np.asarray(b0, dtype=np.float32).reshape(D, 1),
            "b1": np.asarray(b1, dtype=np.float32).reshape(D, 1),
            "b2": b2p,
            "idxt": iw,
            "dcolt": dcol,
            "ncolt": ncol,
            "iotat": iotav,
            "identt": np.eye(64, dtype=np.float32),
        })

    nc = _build_nc(plan)
    trace = bool(int(os.environ.get("GCN_TRACE", "0")))
    if trace:
        _ensure_ntff_hook()
    res = run_bass_kernel_spmd(
        nc, in_maps, list(range(NCORES)),
        trace=trace, tmpdir=os.environ.get("GCN_TRACE_DIR"),
    )
    shards = [res.results[c]["outp"] for c in range(NCORES)]
    out = np.concatenate(shards, axis=0)[:N]
    if res.exec_time_ns is not None:
        kernel.last_exec_time_ns = res.exec_time_ns
    return out.astype(np.float32)


kernel.last_exec_time_ns = None


def _ensure_ntff_hook():
    """Inject the missing antenv.axon_hooks shim + local artifact stash so
    run_bass_kernel_spmd(trace=True) can capture NTFF profiles under axon."""
    import sys
    import types
    import concourse.bass_utils as bu
    if "antenv.axon_hooks" not in sys.modules:
        mod = types.ModuleType("antenv.axon_hooks")
        mod._hook = None

        def set_axon_ntff_profile_hook(h):
            mod._hook = h

        def get_axon_ntff_profile_hook():
            return mod._hook

        mod.set_axon_ntff_profile_hook = set_axon_ntff_profile_hook
        mod.get_axon_ntff_profile_hook = get_axon_ntff_profile_hook
        sys.modules["antenv.axon_hooks"] = mod
        try:
            from trn_agent_boot.trn_boot import _ntff_profile_via_ctypes
            mod._hook = _ntff_profile_via_ctypes("/opt/axon/libaxon_pjrt.so")
        except Exception as e:
            print("ntff hook setup failed:", e)
    bu.upload_artifacts = lambda tmpdir: f"local:{tmpdir}"


# revision 29
# speedup vs baseline: 1.1072x; 1.1072x over previous
"""GCN encoder (3-layer) Trainium2 kernel, 8-core SPMD.

Strategy:
  out = A @ (X @ W) per layer, A = normalized adjacency (fixed across layers).
  - Nodes sharded by contiguous id range across 8 cores (dst partition).
  - Edge slots are region-packed per (window-group, src-chunk): each window's
    slot count is the max edge count over the 8 cores (SPMD shapes), regions
    rounded to 128. 128-slot blocks straddling window boundaries get one
    matmul per overlapped window with a host-zero-masked one-hot tile.
  - Layer 0: the X rows are pre-gathered on host in slot order and streamed
    (transposed) as [128, TOTSLOT] bf16; X@W0 is computed per 128-slot block
    on PE into PSUM, copied into the reg-block layout by DVE. No runtime
    gather and no AllGather for layer 0.
  - Layers 1-2: dense transform fused into the previous layer's window loop,
    AllGather of the [NTAB,128] bf16 table, then per (group,chunk) a
    dma_gather (4 SWDGE queues, one per chunk) pulls slot rows; aggregation
    is a one-hot matmul segment-sum into PSUM (output [feat, dst-window]).
  - Table rows are laid out slice-major ([slice][core][row]) so the
    AllGather is split into 4 slices, each overlapping the tail of the
    producing layer's window loop.
  - Layer 3 output reordered: out = (A @ (X2 @ W2pad)) + b2, W2 zero-padded
    64->128 so gather rows stay 256B.
All graph structure is computed from the inputs at call time and baked into
the compiled program; counts are maxed across cores so all 8 cores run one
SPMD program.
"""

import math
import os
import numpy as np
from contextlib import ExitStack

from ml_dtypes import bfloat16

import concourse.bass as bass
import concourse.bacc as bacc
import concourse.mybir as mybir
import concourse.tile as tile
from concourse.bass_utils import run_bass_kernel_spmd
from concourse import library_config

F32 = mybir.dt.float32
BF16 = mybir.dt.bfloat16
I16 = mybir.dt.int16

NCORES = 8
D = 128          # feature width (layers 0/1/2 input, gather row width)
DOUT = 64        # final output width
WIN = 48         # dst nodes per window (one-hot width, psum free dim)
NCHUNK = 4       # src chunks (int16 gather index range)
GRP = 8          # windows per group (gather-call granularity)
NSLICE = 4       # allgather slices (groups per slice = ceil(NG/NSLICE))


class Plan:
    """Host-derived, core-invariant schedule + per-core tensor data."""

    def __init__(self, n_nodes, edge_src, edge_dst, edge_norm):
        self.N = n_nodes
        self.WPC = math.ceil(n_nodes / (NCORES * WIN))   # windows per core
        self.SH = self.WPC * WIN                         # table rows per core
        self.NTAB = NCORES * self.SH
        self.NG = math.ceil(self.WPC / GRP)
        self.groups = [list(range(g * GRP, min((g + 1) * GRP, self.WPC)))
                       for g in range(self.NG)]

        # allgather slices == src chunks: contiguous runs of groups;
        # slice-major table rows. Each slice is its own Shared tensor and
        # the int16 gather index is relative to the slice start.
        gps = self.NG // NSLICE                          # groups per slice
        assert NSLICE == NCHUNK
        bounds = [0] + [gps * (s + 1) for s in range(NSLICE - 1)] + [self.NG]
        self.slices = [list(range(bounds[s], bounds[s + 1]))
                       for s in range(NSLICE)]
        self.slice_of_group = {}
        srows = []
        for si, gs in enumerate(self.slices):
            rows = sum(len(self.groups[g]) * WIN for g in gs)
            srows.append(rows)
            for g in gs:
                self.slice_of_group[g] = si
        self.srows = srows                               # local rows per slice
        self.sstart = np.concatenate([[0], np.cumsum(srows)[:-1]]).astype(np.int64)
        self.tabsize = [r * NCORES for r in srows]       # table rows per slice
        assert all(t <= 32768 for t in self.tabsize), self.tabsize
        tabstart = np.concatenate([[0], np.cumsum(self.tabsize)[:-1]])
        self.tabstart = tabstart.astype(np.int64)        # table row of slice s

        # R[node] = table row (slice-major: [slice][core][row-in-slice])
        node = np.arange(self.NTAB, dtype=np.int64)
        k = node // self.SH
        loc = node % self.SH
        sidx = np.searchsorted(self.sstart, loc, side="right") - 1
        off = loc - self.sstart[sidx]
        L = np.array(srows, dtype=np.int64)
        self.R = self.tabstart[sidx] + k * L[sidx] + off

        core = edge_dst // self.SH
        wloc = (edge_dst % self.SH) // WIN
        dloc = edge_dst % WIN
        srow = self.R[edge_src]
        ch = (np.searchsorted(self.tabstart, srow, side="right") - 1).astype(np.int64)

        # per-(core,window,chunk) counts -> per-(window,chunk) SPMD max
        cell = (core * self.WPC + wloc) * NCHUNK + ch
        counts = np.bincount(cell, minlength=NCORES * self.WPC * NCHUNK)
        self.counts = counts.reshape(NCORES, self.WPC, NCHUNK)
        maxc = self.counts.max(axis=0)                   # [WPC, NCHUNK]

        # slot layout: for g: for ch: for w in g: maxc[w,ch] slots;
        # each (g,ch) region padded to a multiple of 128.
        self.wstart = {}       # (w, c) -> global slot start
        self.rstart = [[0] * NCHUNK for _ in range(self.NG)]
        self.rslot = [[0] * NCHUNK for _ in range(self.NG)]
        off = 0
        for g, ws in enumerate(self.groups):
            for c in range(NCHUNK):
                r0 = off
                for w in ws:
                    self.wstart[(w, c)] = off
                    off += int(maxc[w, c])
                off = -(-off // 128) * 128
                self.rstart[g][c] = r0
                self.rslot[g][c] = off - r0
        self.TOTSLOT = off
        assert self.TOTSLOT % 128 == 0

        # matmul schedule: per group, per window, list of (c, j_local, mm)
        self.sched = []        # [g][w-local] -> list of (c, j_local, mm)
        self.gmm0 = []         # per-group first mm index
        self.mmof = {}         # (w, c, j_local) -> mm
        mm = 0
        for g, ws in enumerate(self.groups):
            self.gmm0.append(mm)
            gs = []
            for w in ws:
                entries = []
                for c in range(NCHUNK):
                    n = int(maxc[w, c])
                    if n == 0:
                        continue
                    s0 = self.wstart[(w, c)]
                    j0 = (s0 - self.rstart[g][c]) // 128
                    j1 = (s0 + n - 1 - self.rstart[g][c]) // 128
                    for j in range(j0, j1 + 1):
                        entries.append((c, j, mm))
                        self.mmof[(w, c, j)] = mm
                        mm += 1
                gs.append(entries)
            self.sched.append(gs)
        self.NMM = mm
        self.gmm0.append(mm)

        self.maxc = maxc
        self.edge_core = core
        self.edge_w = wloc
        self.edge_d = dloc
        self.edge_ch = ch
        self.edge_srow = srow
        self.edge_norm = edge_norm

    def core_slots(self, k):
        """Per-core slot assignment: returns (slot, srow, w, ch, d, norm)."""
        sel = np.nonzero(self.edge_core == k)[0]
        w = self.edge_w[sel]
        ch = self.edge_ch[sel]
        d = self.edge_d[sel]
        srow = self.edge_srow[sel]
        norm = self.edge_norm[sel]
        order = np.lexsort((ch, w))
        w, ch, d, srow, norm = w[order], ch[order], d[order], srow[order], norm[order]
        bucket = w * NCHUNK + ch
        nb = self.WPC * NCHUNK
        bc = np.bincount(bucket, minlength=nb)
        starts = np.concatenate([[0], np.cumsum(bc)[:-1]])
        rank = np.arange(len(sel)) - starts[bucket]
        base = np.empty(nb, dtype=np.int64)
        for wi in range(self.WPC):
            for ci in range(NCHUNK):
                base[wi * NCHUNK + ci] = self.wstart.get((wi, ci), 0)
        slot = base[bucket] + rank
        return slot, srow, w, ch, d, norm

    def core_tensors(self, k):
        """Build idx table + one-hot table for core k."""
        slot, srow, w, ch, d, norm = self.core_slots(k)

        idxs = np.zeros(self.TOTSLOT, dtype=np.int16)
        idxs[slot] = (srow - self.tabstart[ch]).astype(np.int16)
        iw = idxs.reshape(-1, 16).T.copy()            # [16, TOTSLOT//16]
        iw = np.tile(iw, (8, 1))                      # replicate to 128 parts

        # compact one-hot: per (slot-row, mm) a (d, norm) pair; the kernel
        # expands to [128, nmm, WIN] on DVE via is_equal(iota, d) * norm.
        gidx = w // GRP
        rs = np.array([[self.rstart[g][c] for c in range(NCHUNK)]
                       for g in range(self.NG)], dtype=np.int64)
        jloc = (slot - rs[gidx, ch]) // 128
        mmv = np.empty(len(slot), dtype=np.int64)
        for i in range(len(slot)):
            mmv[i] = self.mmof[(int(w[i]), int(ch[i]), int(jloc[i]))]
        dcol = np.zeros((128, self.NMM), dtype=np.float32)
        ncol = np.zeros((128, self.NMM), dtype=np.float32)
        dcol[slot % 128, mmv] = d
        ncol[slot % 128, mmv] = norm
        return iw, dcol.astype(bfloat16), ncol.astype(bfloat16)


def _build_nc(plan, enable_asserts=False):
    p = plan
    no_coll = bool(int(os.environ.get("GCN_NO_COLL", "0")))
    nc = bacc.Bacc(
        "TRN2",
        target_bir_lowering=False,
        debug=False,
        enable_asserts=enable_asserts,
        num_devices=NCORES,
        num_swdge_queues=4,
        dynamic_dma_scratch_size=16384,
    )
    xgT = nc.dram_tensor("xgT", [D, p.TOTSLOT], BF16, kind="ExternalInput")
    w0 = nc.dram_tensor("w0", [D, D], BF16, kind="ExternalInput")
    w1 = nc.dram_tensor("w1", [D, D], BF16, kind="ExternalInput")
    w2 = nc.dram_tensor("w2", [D, D], BF16, kind="ExternalInput")
    b0 = nc.dram_tensor("b0", [D, 1], F32, kind="ExternalInput")
    b1 = nc.dram_tensor("b1", [D, 1], F32, kind="ExternalInput")
    b2 = nc.dram_tensor("b2", [D, 1], F32, kind="ExternalInput")
    idxt = nc.dram_tensor("idxt", [128, p.TOTSLOT // 16], I16, kind="ExternalInput")
    dcolt = nc.dram_tensor("dcolt", [128, p.NMM], BF16, kind="ExternalInput")
    ncolt = nc.dram_tensor("ncolt", [128, p.NMM], BF16, kind="ExternalInput")
    iotat = nc.dram_tensor("iotat", [128, WIN], BF16, kind="ExternalInput")
    identt = nc.dram_tensor("identt", [64, 64], F32, kind="ExternalInput")
    outp = nc.dram_tensor("outp", [p.SH, DOUT], F32, kind="ExternalOutput")

    with tile.TileContext(nc) as tc, ExitStack() as ctx:
        nc.gpsimd.load_library(library_config.mlp)
        sb = ctx.enter_context(tc.tile_pool(name="sb", bufs=2))
        sbg = ctx.enter_context(tc.tile_pool(name="sbg", bufs=5))
        sbo = ctx.enter_context(tc.tile_pool(name="sbo", bufs=2))
        sbh = ctx.enter_context(tc.tile_pool(name="sbh", bufs=2))
        sbp = ctx.enter_context(tc.tile_pool(name="sbp", bufs=1))
        ps = ctx.enter_context(tc.tile_pool(name="ps", bufs=2, space="PSUM"))
        psa = ctx.enter_context(tc.tile_pool(name="psa", bufs=3, space="PSUM"))
        psd = ctx.enter_context(tc.tile_pool(name="psd", bufs=3, space="PSUM"))
        dram = ctx.enter_context(tc.tile_pool(name="dram", bufs=1, space="DRAM"))

        t_outs = [dram.tile([r, D], BF16, name=f"t_out{s}")
                  for s, r in enumerate(p.srows)]
        t_tabs = [[dram.tile([p.tabsize[s], D], BF16, addr_space="Shared",
                             name=f"t_tab{i}_{s}") for s in range(len(p.srows))]
                  for i in range(2)]

        # persistent tiles
        idx_sb = sbp.tile([128, p.TOTSLOT // 16], I16)
        nc.sync.dma_start(out=idx_sb[:], in_=idxt[:])
        wt = []
        for wsrc in (w0, w1, w2):
            w_sb = sbp.tile([D, D], BF16, name=f"w_{wsrc.name}")
            nc.sync.dma_start(out=w_sb[:], in_=wsrc[:])
            wt.append(w_sb)
        bt = []
        for bsrc in (b0, b1, b2):
            b_sb = sbp.tile([D, 1], F32, name=f"b_{bsrc.name}")
            nc.sync.dma_start(out=b_sb[:], in_=bsrc[:])
            bt.append(b_sb)
        ident = sbp.tile([64, 64], F32)
        nc.sync.dma_start(out=ident[:], in_=identt[:])
        iota = sbp.tile([128, 1, WIN], BF16)
        nc.sync.dma_start(out=iota[:], in_=iotat[:])

        def do_allgather(s, layer):
            src_tile = t_outs[s]
            dst = t_tabs[layer][s]
            if no_coll:
                nc.sync.dma_start(out=dst[:p.srows[s], :], in_=src_tile[:])
                return
            nc.gpsimd.collective_compute(
                "AllGather",
                mybir.AluOpType.bypass,
                replica_groups=[list(range(NCORES))],
                ins=[src_tile.opt()],
                outs=[dst.opt()],
            )

        for layer in range(3):
            t_tab = t_tabs[layer - 1] if layer > 0 else None
            for g, ws in enumerate(p.groups):
                si = p.slice_of_group[g]
                regs = []
                for c in range(NCHUNK):
                    nslots = p.rslot[g][c]
                    nb = nslots // 128
                    pool = sbh if layer == 0 else sbg
                    reg = pool.tile([128, nb, 128], BF16,
                                    tag=f"reg{c}", name=f"reg_{layer}_{g}_{c}")
                    if layer == 0:
                        s0 = p.rstart[g][c]
                        xg = sb.tile([128, nslots], BF16, tag="xg",
                                     name=f"xg_{g}_{c}")
                        nc.sync.dma_start(out=xg[:], in_=xgT[:, s0:s0 + nslots])
                        for j0 in range(0, nb, 4):
                            j1 = min(j0 + 4, nb)
                            pd = psd.tile([128, 4, D], F32, tag="pdense",
                                          name=f"pd_{g}_{c}_{j0}")
                            for j in range(j0, j1):
                                nc.tensor.matmul(
                                    out=pd[:, j - j0, :],
                                    lhsT=xg[:, j * 128:(j + 1) * 128],
                                    rhs=wt[0][:], start=True, stop=True)
                            nc.scalar.activation(
                                out=reg[:, j0:j1, :], in_=pd[:, :j1 - j0, :],
                                func=mybir.ActivationFunctionType.Copy)
                    else:
                        ic0 = p.rstart[g][c] // 16
                        nc.gpsimd.dma_gather(
                            out_ap=reg[:],
                            in_ap=t_tab[c][:],
                            idxs_ap=idx_sb[:, ic0:ic0 + nslots // 16],
                            num_idxs=nslots,
                            num_idxs_reg=nslots,
                            elem_size=D,
                            single_packet=False,
                            queue_num=c,
                        )
                    regs.append(reg)
                mm0, mm1 = p.gmm0[g], p.gmm0[g + 1]
                nmm = mm1 - mm0
                dn_g = sbo.tile([128, 2, nmm], BF16, tag="dn",
                                name=f"dn_{layer}_{g}")
                nc.sync.dma_start(out=dn_g[:, 0, :], in_=dcolt[:, mm0:mm1])
                nc.sync.dma_start(out=dn_g[:, 1, :], in_=ncolt[:, mm0:mm1])
                oh_g = sbo.tile([128, nmm, WIN], BF16, tag="oh",
                                name=f"oh_{layer}_{g}")
                nc.vector.tensor_tensor(
                    out=oh_g[:],
                    in0=iota[:].broadcast_to([128, nmm, WIN]),
                    in1=dn_g[:, 0, :].broadcast_to([128, nmm, WIN]),
                    op=mybir.AluOpType.is_equal)
                nc.vector.tensor_tensor(
                    out=oh_g[:], in0=oh_g[:],
                    in1=dn_g[:, 1, :].broadcast_to([128, nmm, WIN]),
                    op=mybir.AluOpType.mult)
                if layer < 2:
                    tt2g = sbo.tile([WIN, GRP, D], BF16, tag="tnext",
                                    name=f"tnext_{layer}_{g}")
                else:
                    oo_g = sbo.tile([WIN, GRP, DOUT], F32, tag="oo",
                                    name=f"oo_{g}")
                for wi, w in enumerate(ws):
                    entries = p.sched[g][wi]
                    pT = psa.tile([D, WIN], F32, tag="pagg", name=f"pagg_{layer}_{w}")
                    nmmw = len(entries)
                    for k, (c, j, mmi) in enumerate(entries):
                        nc.tensor.matmul(
                            out=pT[:],
                            lhsT=regs[c][:, j, :],
                            rhs=oh_g[:, mmi - mm0, :],
                            start=(k == 0),
                            stop=(k == nmmw - 1),
                        )
                    if layer < 2:
                        xnT = sb.tile([D, WIN], BF16, tag="xn",
                                      name=f"xn_{layer}_{w}")
                        nc.scalar.activation(
                            out=xnT[:], in_=pT[:],
                            func=mybir.ActivationFunctionType.Relu,
                            bias=bt[layer][:],
                        )
                        pt2 = ps.tile([WIN, D], F32, tag="pnext",
                                      name=f"pnext_{layer}_{w}")
                        nc.tensor.matmul(out=pt2[:], lhsT=xnT[:],
                                         rhs=wt[layer + 1][:],
                                         start=True, stop=True)
                        nc.scalar.activation(
                            out=tt2g[:, wi, :], in_=pt2[:],
                            func=mybir.ActivationFunctionType.Copy)
                    else:
                        x3 = sb.tile([D, WIN], F32, tag="x3", name=f"x3_{w}")
                        nc.vector.tensor_scalar(
                            out=x3[:], in0=pT[:], scalar1=bt[2][:], scalar2=None,
                            op0=mybir.AluOpType.add)
                        po = ps.tile([WIN, DOUT], F32, tag="pnext",
                                     name=f"pout_{w}")
                        nc.tensor.matmul(out=po[:], lhsT=x3[:DOUT, :],
                                         rhs=ident[:], start=True, stop=True)
                        nc.scalar.activation(
                            out=oo_g[:, wi, :], in_=po[:],
                            func=mybir.ActivationFunctionType.Copy)
                if layer < 2:
                    # batched table write for this group's WIN*len(ws) rows
                    g0 = p.slices[si][0]
                    row0 = sum(len(p.groups[gg]) * WIN
                               for gg in p.slices[si] if gg < g)
                    nw = len(ws)
                    nc.sync.dma_start(
                        out=t_outs[si][row0:row0 + nw * WIN, :].rearrange(
                            "(w q) d -> q w d", q=WIN),
                        in_=tt2g[:, :nw, :])
                    if g == p.slices[si][-1]:
                        do_allgather(si, layer)
                else:
                    r0 = ws[0] * WIN
                    nw = len(ws)
                    nc.sync.dma_start(
                        out=outp[r0:r0 + nw * WIN, :].rearrange(
                            "(w q) f -> q w f", q=WIN),
                        in_=oo_g[:, :nw, :])
    nc.compile()
    return nc


def _prep(x, edge_index, edge_weight):
    N = x.shape[0]
    src = np.asarray(edge_index[0], dtype=np.int64)
    dst = np.asarray(edge_index[1], dtype=np.int64)
    w = np.asarray(edge_weight, dtype=np.float64)
    loop = np.arange(N, dtype=np.int64)
    src_f = np.concatenate([src, loop])
    dst_f = np.concatenate([dst, loop])
    w_f = np.concatenate([w, np.ones(N)])
    deg = np.bincount(dst_f, weights=w_f, minlength=N)
    dis = np.where(deg > 0, 1.0 / np.sqrt(np.where(deg > 0, deg, 1.0)), 0.0)
    norm = (dis[src_f] * w_f * dis[dst_f]).astype(np.float32)
    return src_f.astype(np.int64), dst_f.astype(np.int64), norm


def kernel(x, edge_index, edge_weight, W0, b0, W1, b1, W2, b2):
    x = np.asarray(x, dtype=np.float32)
    N = x.shape[0]
    src_f, dst_f, norm = _prep(x, edge_index, edge_weight)
    plan = Plan(N, src_f, dst_f, norm)

    W2p = np.zeros((D, D), dtype=np.float32)
    W2p[:, :DOUT] = np.asarray(W2)
    b2p = np.zeros((D, 1), dtype=np.float32)
    b2p[:DOUT, 0] = np.asarray(b2)

    # X rows in table-row space (R is a permutation of [0, NTAB))
    xrows = np.zeros((plan.NTAB, D), dtype=np.float32)
    xrows[plan.R[:N]] = x

    iotav = np.tile(np.arange(WIN, dtype=np.float32), (128, 1)).astype(bfloat16)
    in_maps = []
    for c in range(NCORES):
        iw, dcol, ncol = plan.core_tensors(c)
        slot, srow, _, _, _, _ = plan.core_slots(c)
        srcs = np.zeros(plan.TOTSLOT, dtype=np.int64)
        srcs[slot] = srow
        xg = np.ascontiguousarray(xrows[srcs].T).astype(bfloat16)
        in_maps.append({
            "xgT": xg,
            "w0": np.asarray(W0).astype(bfloat16),
            "w1": np.asarray(W1).astype(bfloat16),
            "w2": W2p.astype(bfloat16),
            "b0": np.asarray(b0, dtype=np.float32).reshape(D, 1),
            "b1": np.asarray(b1, dtype=np.float32).reshape(D, 1),
            "b2": b2p,
            "idxt": iw,
            "dcolt": dcol,
            "ncolt": ncol,
            "iotat": iotav,
            "identt": np.eye(64, dtype=np.float32),
        })

    nc = _build_nc(plan)
    trace = bool(int(os.environ.get("GCN_TRACE", "0")))
    if trace:
        _ensure_ntff_hook()
    res = run_bass_kernel_spmd(
        nc, in_maps, list(range(NCORES)),
        trace=trace, tmpdir=os.environ.get("GCN_TRACE_DIR"),
    )
    shards = [res.results[c]["outp"] for c in range(NCORES)]
    out = np.concatenate(shards, axis=0)[:N]
    if res.exec_time_ns is not None:
        kernel.last_exec_time_ns = res.exec_time_ns
    return out.astype(np.float32)


kernel.last_exec_time_ns = None


def _ensure_ntff_hook():
    """Inject the missing antenv.axon_hooks shim + local artifact stash so
    run_bass_kernel_spmd(trace=True) can capture NTFF profiles under axon."""
    import sys
    import types
    import concourse.bass_utils as bu
    if "antenv.axon_hooks" not in sys.modules:
        mod = types.ModuleType("antenv.axon_hooks")
        mod._hook = None

        def set_axon_ntff_profile_hook(h):
            mod._hook = h

        def get_axon_ntff_profile_hook():
            return mod._hook

        mod.set_axon_ntff_profile_hook = set_axon_ntff_profile_hook
        mod.get_axon_ntff_profile_hook = get_axon_ntff_profile_hook
        sys.modules["antenv.axon_hooks"] = mod
        try:
            from trn_agent_boot.trn_boot import _ntff_profile_via_ctypes
            mod._hook = _ntff_profile_via_ctypes("/opt/axon/libaxon_pjrt.so")
        except Exception as e:
            print("ntff hook setup failed:", e)
    bu.upload_artifacts = lambda tmpdir: f"local:{tmpdir}"
